# revision 30
# baseline (speedup 1.0000x reference)
"""Trainium2 Bass kernel for a 3D windowed-attention transformer block.

Strategy: data-parallel over the 16 attention windows (2 windows/core x 8 cores).
All tensors on device live in "C-layout": [channels(partitions), tokens(free)].
Host folds LayerNorm gains into the matmul weights, builds an augmented qkv
weight whose extra output rows give the decomposed rel-pos bias terms, and the
attention bias is realized by extending the QK contraction with one-hot key
position rows. Softmax runs without max-subtraction (logits are O(1) by
construction) and its normalization is deferred past the PV matmul via an
appended ones-column in V (row 64 of the PV output accumulates the denominator).
"""

import ml_dtypes
import numpy as np

import concourse.bass as bass
import concourse.mybir as mybir
import concourse.tile as tile
from concourse import bacc

DIM = 768
NH = 12
HD = 64
WS = 8
NTOK = 1024          # tokens per core (2 windows x 512)
NWIN = 2
KT = DIM // 128      # 6 k-tiles over channels
SCALE = HD ** -0.5
EPS = 1e-5
F32 = mybir.dt.float32
F32R = mybir.dt.float32r
BF16 = mybir.dt.bfloat16


# ---------------------------------------------------------------------------
# device program
# ---------------------------------------------------------------------------

def _emit_ln(nc, tc, stack, xs, xhat, consts):
    """Standardize along channels: xhat = (x - mean)/sqrt(var+eps). C-layout.

    xs: fp32 input tiles; xhat: BF16 output tiles (pre-rounded for matmul rhs).
    Channel reductions go through the PE with bf16 operands, accumulating fp32.
    """
    from contextlib import ExitStack
    with ExitStack() as ctx:
        stat_ps = ctx.enter_context(tc.tile_pool(name="ln_stat_ps", bufs=2, space="PSUM"))
        bc_ps = ctx.enter_context(tc.tile_pool(name="ln_bc_ps", bufs=2, space="PSUM"))
        sq_pool = ctx.enter_context(tc.tile_pool(name="ln_sq", bufs=2))
        xb_pool = ctx.enter_context(tc.tile_pool(name="ln_xb", bufs=2))
        st_pool = ctx.enter_context(tc.tile_pool(name="ln_st", bufs=2))
        for ch in range(2):
            cols = bass.ds(ch * 512, 512)
            ps_s = stat_ps.tile([1, 512], F32, tag="ps_s")
            ps_q = stat_ps.tile([1, 512], F32, tag="ps_q")
            for k in range(KT):
                xb = xb_pool.tile([128, 512], BF16, tag="xb")
                nc.scalar.copy(xb, xs[k][:, cols])
                sq = sq_pool.tile([128, 512], BF16, tag="sq")
                nc.vector.tensor_mul(sq, xs[k][:, cols], xs[k][:, cols])
                nc.tensor.matmul(ps_s, consts["ones_col"], xb,
                                 start=(k == 0), stop=(k == KT - 1))
                nc.tensor.matmul(ps_q, consts["ones_col"], sq,
                                 start=(k == 0), stop=(k == KT - 1))
            mean = st_pool.tile([1, 512], BF16, tag="mean")
            ex2 = st_pool.tile([1, 512], F32, tag="ex2")
            nc.scalar.mul(mean, ps_s, 1.0 / DIM)
            nc.scalar.mul(ex2, ps_q, 1.0 / DIM)
            var = st_pool.tile([1, 512], F32, tag="var")
            nc.vector.tensor_mul(var, mean, mean)
            nc.vector.tensor_sub(var, ex2, var)
            rstd = st_pool.tile([1, 512], F32, tag="rstd")
            nc.scalar.activation(rstd, var, mybir.ActivationFunctionType.Sqrt,
                                 bias=consts["eps"][0:1, 0:1], scale=1.0)
            rstd_b = st_pool.tile([1, 512], BF16, tag="rstd_b")
            nc.vector.reciprocal(rstd_b, rstd)
            bc_m = bc_ps.tile([128, 512], F32, tag="bc_m")
            bc_r = bc_ps.tile([128, 512], F32, tag="bc_r")
            nc.tensor.matmul(bc_m, consts["ones_row"], mean, start=True, stop=True)
            nc.tensor.matmul(bc_r, consts["ones_row"], rstd_b, start=True, stop=True)
            for k in range(KT):
                sb = sq_pool.tile([128, 512], F32, tag="sb")
                nc.vector.tensor_sub(sb, xs[k][:, cols], bc_m)
                nc.vector.tensor_mul(xhat[k][:, cols], sb, bc_r)


def _emit(nc, tc, d):
    """Emit the whole per-core program. d: dict of DRAM APs."""
    from contextlib import ExitStack

    Ident = mybir.ActivationFunctionType.Identity
    with ExitStack() as top:
        top.enter_context(nc.allow_low_precision(
            reason="bf16 matmul operands are intentional; accumulation stays fp32"))
        consts_pool = top.enter_context(tc.tile_pool(name="consts", bufs=1))
        ones_col = consts_pool.tile([128, 1], BF16)
        nc.vector.memset(ones_col, 1.0)
        ones_row = consts_pool.tile([1, 128], BF16)
        nc.vector.memset(ones_row, 1.0)
        ones_r64 = consts_pool.tile([1, 64], BF16)
        nc.vector.memset(ones_r64, 1.0)
        eps = consts_pool.tile([1, 1], F32)
        nc.vector.memset(eps, EPS)
        consts = dict(ones_col=ones_col, ones_row=ones_row, eps=eps)

        baug = consts_pool.tile([128, 18], F32)
        nc.sync.dma_start(out=baug, in_=d["baug"])
        bp = consts_pool.tile([128, 6], F32)
        nc.sync.dma_start(out=bp, in_=d["bp"])
        b1 = consts_pool.tile([128, 24], F32)
        nc.sync.dma_start(out=b1, in_=d["b1"])
        b2 = consts_pool.tile([128, 6], F32)
        nc.sync.dma_start(out=b2, in_=d["b2"])
        bv = consts_pool.tile([1, 768], BF16)
        nc.sync.dma_start(out=bv, in_=d["bv"])
        sel_sb = consts_pool.tile([128, 8 * 128], BF16)
        nc.sync.dma_start(out=sel_sb, in_=d["sel"])

        # attention output, lives until proj consumes it (kept to end for LIFO)
        ctx_pool = top.enter_context(tc.tile_pool(name="ctxT", bufs=KT))
        ctxT = [ctx_pool.tile([128, NTOK], BF16, tag="ctxT", name=f"ctxT{i}") for i in range(KT)]

        # persistent activation tensors for the attention stage (phases 1-3)
        qh_cm = tc.tile_pool(name="qhat", bufs=NH)
        kh_cm = tc.tile_pool(name="khat", bufs=NH)
        vh_cm = tc.tile_pool(name="vhat", bufs=8)
        qh_pool = qh_cm.__enter__()
        kh_pool = kh_cm.__enter__()
        vh_pool = vh_cm.__enter__()
        qhat = [qh_pool.tile([128, NTOK], BF16, tag="qhat", name=f"qhat{i}") for i in range(NH)]
        khat = [kh_pool.tile([128, NTOK], BF16, tag="khat", name=f"khat{i}") for i in range(NH)]
        vhat = [vh_pool.tile([128, NH * 65], BF16, tag="vhat", name=f"vhat{i}") for i in range(8)]

        # one-hot key-position rows of khat (rows 0-63, incl. zero gaps)
        for h in range(NH):
            nc.sync.dma_start(out=khat[h][0:64, :], in_=d["e2"])
        # zero qhat so never-written rows multiply khat zeros as 0 (not NaN)
        for h in range(NH):
            nc.vector.memset(qhat[h], 0.0)
        # ones columns of vhat (col 64 of each head block)
        for t in range(8):
            vcols = vhat[t].rearrange("p (h c) -> p h c", c=65)
            nc.vector.memset(vcols[:, :, 64:65], 1.0)

        # ---- phases 1-3: LN1, v, then per-head-pair qkv+attention interleave ----
        with ExitStack() as ph:
            xh_pool = ph.enter_context(tc.tile_pool(name="xhat", bufs=KT))
            xhat = [xh_pool.tile([128, NTOK], BF16, tag="xh", name=f"xh{i}") for i in range(KT)]
            with ExitStack() as lnph:
                xt_pool = lnph.enter_context(tc.tile_pool(name="xt", bufs=KT))
                xs = [xt_pool.tile([128, NTOK], F32, tag="xt", name=f"xt{i}") for i in range(KT)]
                for k in range(KT):
                    nc.sync.dma_start(out=xs[k], in_=d["xT"][k * 128:(k + 1) * 128, :])
                _emit_ln(nc, tc, lnph, xs, xhat, consts)

            # all qkv weights resident in SBUF: no weight DMA inside the loop
            wq_pool = ph.enter_context(tc.tile_pool(name="wqfull", bufs=KT))
            wqs = [wq_pool.tile([128, 2304], BF16, tag="wqf", name=f"wqf{i}") for i in range(KT)]
            for k in range(KT):
                nc.sync.dma_start(out=wqs[k], in_=d["waug"][k * 128:(k + 1) * 128, :])

            # v projection first (vhat only depends on xhat)
            wv_pool = ph.enter_context(tc.tile_pool(name="wv", bufs=KT))
            wvs = [wv_pool.tile([128, 768], BF16, tag="wv", name=f"wv{i}") for i in range(KT)]
            for k in range(KT):
                nc.sync.dma_start(out=wvs[k], in_=d["wv"][k * 128:(k + 1) * 128, :])
            with ExitStack() as vph:
                v_ps = vph.enter_context(tc.tile_pool(name="v_ps", bufs=2, space="PSUM"))
                for t in range(8):
                    for nch in range(2):
                        pv = v_ps.tile([128, 384], F32, tag="vps")
                        for k in range(KT):
                            nc.tensor.matmul(pv, xhat[k][:, t * 128:(t + 1) * 128],
                                             wvs[k][:, nch * 384:(nch + 1) * 384],
                                             start=(k == 0), stop=False)
                        nc.tensor.matmul(pv, ones_row, bv[0:1, nch * 384:(nch + 1) * 384],
                                         start=False, stop=True)
                        vh_r = vhat[t].rearrange("p (h c) -> p h c", c=65)
                        nc.vector.tensor_copy(vh_r[:, nch * 6:(nch + 1) * 6, 0:64],
                                              pv.rearrange("p (h c) -> p h c", c=64))

            qkv_ps = ph.enter_context(tc.tile_pool(name="qkv_ps", bufs=2, space="PSUM"))
            pb_ps = ph.enter_context(tc.tile_pool(name="pb_ps", bufs=2, space="PSUM"))
            psb_pool = ph.enter_context(tc.tile_pool(name="psb", bufs=3))
            s_ps = ph.enter_context(tc.tile_pool(name="s_ps", bufs=2, space="PSUM"))
            c_ps = ph.enter_context(tc.tile_pool(name="c_ps", bufs=1, space="PSUM"))
            r_ps = ph.enter_context(tc.tile_pool(name="r_ps", bufs=1, space="PSUM"))
            pt_pool = ph.enter_context(tc.tile_pool(name="ptile", bufs=8))
            dn_pool = ph.enter_context(tc.tile_pool(name="dn", bufs=4))

            for hp in range(6):
                for m in (hp, 6 + hp, 12 + hp):
                    for ch in range(2):
                        pt = qkv_ps.tile([128, 512], F32, tag="qkvps",
                                         name=f"qkvps_{m}_{ch}")
                        for k in range(KT):
                            nc.tensor.matmul(
                                pt, wqs[k][:, m * 128:(m + 1) * 128],
                                xhat[k][:, ch * 512:(ch + 1) * 512],
                                start=(k == 0), stop=(k == KT - 1))
                        cols = bass.ds(ch * 512, 512)
                        nc.scalar.activation(pt, pt, Ident,
                                             bias=baug[:, m:m + 1], scale=1.0)
                        if m < 6:        # q rows (scaled): heads 2m, 2m+1
                            nc.scalar.copy(qhat[2 * m][64:128, cols], pt[0:64, :])
                            nc.scalar.copy(qhat[2 * m + 1][64:128, cols], pt[64:128, :])
                        elif m < 12:     # k rows: heads 2(m-6), 2(m-6)+1
                            nc.scalar.copy(khat[2 * (m - 6)][64:128, cols], pt[0:64, :])
                            nc.scalar.copy(khat[2 * (m - 6) + 1][64:128, cols], pt[64:128, :])
                        else:            # P rows -> rel-pos B rows of qhat
                            psb = psb_pool.tile([128, 512], BF16, tag="psb",
                                                name=f"psb_{m}_{ch}")
                            nc.scalar.copy(psb, pt)
                            for hh in range(2):
                                h = 2 * (m - 12) + hh
                                base = hh * 64
                                q_r = qhat[h].rearrange("p (w a b c) -> p w a b c",
                                                        w=NWIN, a=8, b=8)
                                # d-table -> qhat rows 8-15 via direct SBUF DMA
                                for dlt in range(8):
                                    srow = base + 7 - dlt
                                    nc.gpsimd.dma_start(
                                        out=qhat[h][8:16, ch * 512 + dlt * 64:
                                                    ch * 512 + (dlt + 1) * 64],
                                        in_=psb[srow:srow + 8, dlt * 64:(dlt + 1) * 64])
                                # h/w tables via one-hot selector matmuls
                                for ti in (1, 2):
                                    trow = 0 if ti == 1 else 32
                                    for g in range(2):
                                        v = hh * 4 + (ti - 1) * 2 + g
                                        pbm = pb_ps.tile([128, 512], F32, tag="pbm",
                                                         name=f"pbm_{m}_{ch}_{v}")
                                        nc.tensor.matmul(
                                            pbm, sel_sb[:, v * 128:(v + 1) * 128],
                                            psb, start=True, stop=True)
                                        pbm_r = pbm.rearrange(
                                            "p (a b c) -> p a b c", a=8, b=8)
                                        for dp in range(4):
                                            dlt = g * 4 + dp
                                            srows = slice(dp * 32, dp * 32 + 8)
                                            if ti == 1:
                                                nc.vector.tensor_copy(
                                                    q_r[trow:trow + 8, ch, :, dlt, :],
                                                    pbm_r[srows, :, dlt, :])
                                            else:
                                                nc.vector.tensor_copy(
                                                    q_r[trow:trow + 8, ch, :, :, dlt],
                                                    pbm_r[srows, :, :, dlt])
                # attention for this head pair
                for h in (2 * hp, 2 * hp + 1):
                    for wi in range(NWIN):
                        qcols = bass.ds(wi * 512, 512)
                        pts_l = []
                        for kt in range(4):
                            ps = s_ps.tile([128, 512], F32, tag="sps")
                            nc.tensor.matmul(
                                ps, khat[h][:, wi * 512 + kt * 128: wi * 512 + (kt + 1) * 128],
                                qhat[h][:, qcols], start=True, stop=True)
                            ptile = pt_pool.tile([128, 512], BF16, tag="pt")
                            nc.scalar.activation(ptile, ps,
                                                 mybir.ActivationFunctionType.Exp)
                            pts_l.append(ptile)
                        pc = c_ps.tile([65, 512], F32, tag="cps")
                        for kt in range(4):
                            nc.tensor.matmul(pc, vhat[wi * 4 + kt][:, h * 65:h * 65 + 65],
                                             pts_l[kt], start=(kt == 0), stop=(kt == 3))
                        rec = dn_pool.tile([1, 512], BF16, tag="rec")
                        nc.vector.reciprocal(rec, pc[64:65, :])
                        pb = r_ps.tile([64, 512], F32, tag="rps")
                        nc.tensor.matmul(pb, ones_r64, rec, start=True, stop=True)
                        bb = dn_pool.tile([64, 512], F32, tag="bb")
                        nc.scalar.copy(bb, pb)
                        nc.vector.tensor_mul(
                            ctxT[h // 2][(h % 2) * 64:(h % 2) * 64 + 64, qcols],
                            pc[0:64, :], bb)

        vh_cm.__exit__(None, None, None)
        kh_cm.__exit__(None, None, None)
        qh_cm.__exit__(None, None, None)

        if PHASE_LIMIT == "attn":
            with ExitStack() as ph:
                o_pool = ph.enter_context(tc.tile_pool(name="odbg", bufs=2))
                for k in range(KT):
                    ot = o_pool.tile([128, NTOK], F32, tag="ot")
                    nc.scalar.copy(ot, ctxT[k])
                    nc.sync.dma_start(out=d["outT"][k * 128:(k + 1) * 128, :], in_=ot)
            return

        # ---- phase 4: proj + residual ----
        x2_pool = top.enter_context(tc.tile_pool(name="x2", bufs=KT))
        x2 = [x2_pool.tile([128, NTOK], F32, tag="x2", name=f"x2_{i}") for i in range(KT)]
        with ExitStack() as ph:
            wp_pool = ph.enter_context(tc.tile_pool(name="wp", bufs=KT))
            xr_pool = ph.enter_context(tc.tile_pool(name="xr", bufs=KT))
            p_ps = ph.enter_context(tc.tile_pool(name="p_ps", bufs=4, space="PSUM"))
            wps = [wp_pool.tile([128, 768], BF16, tag="wp", name=f"wp{i}") for i in range(KT)]
            xr = [xr_pool.tile([128, NTOK], F32, tag="xr", name=f"xr{i}") for i in range(KT)]
            for k in range(KT):
                nc.sync.dma_start(out=wps[k], in_=d["wp"][k * 128:(k + 1) * 128, :])
                nc.sync.dma_start(out=xr[k], in_=d["xT"][k * 128:(k + 1) * 128, :])
            for ch in range(2):
                for m in range(KT):
                    pp = p_ps.tile([128, 512], F32, tag="pps")
                    for k in range(KT):
                        nc.tensor.matmul(pp, wps[k][:, m * 128:(m + 1) * 128],
                                         ctxT[k][:, ch * 512:(ch + 1) * 512],
                                         start=(k == 0), stop=(k == KT - 1))
                    nc.vector.scalar_tensor_tensor(
                        out=x2[m][:, ch * 512:(ch + 1) * 512],
                        in0=pp, scalar=bp[:, m:m + 1],
                        in1=xr[m][:, ch * 512:(ch + 1) * 512],
                        op0=mybir.AluOpType.add, op1=mybir.AluOpType.add)

        # ---- phase 5+6: LN2 + fc1 + gelu ----
        h1_pool = top.enter_context(tc.tile_pool(name="h1", bufs=24))
        h1 = [h1_pool.tile([128, NTOK], BF16, tag="h1", name=f"h1_{i}") for i in range(24)]
        with ExitStack() as ph:
            mh_pool = ph.enter_context(tc.tile_pool(name="mhat", bufs=KT))
            mhat = [mh_pool.tile([128, NTOK], BF16, tag="mh", name=f"mh{i}") for i in range(KT)]
            _emit_ln(nc, tc, ph, x2, mhat, consts)
            f1_ps = ph.enter_context(tc.tile_pool(name="f1_ps", bufs=6, space="PSUM"))
            w1_pool = ph.enter_context(tc.tile_pool(name="w1", bufs=3))
            mgroups = [list(range(s, s + 3)) for s in range(0, 24, 3)]
            for ms in mgroups:
                pts = {}
                for mi, m in enumerate(ms):
                    for ch in range(2):
                        pts[(mi, ch)] = f1_ps.tile([128, 512], F32, tag="f1ps", name=f"f1ps_{m}_{ch}")
                for k in range(KT):
                    w1 = w1_pool.tile([128, 128 * len(ms)], BF16, tag="w1")
                    nc.sync.dma_start(
                        out=w1, in_=d["w1"][k * 128:(k + 1) * 128,
                                            ms[0] * 128:(ms[-1] + 1) * 128])
                    for mi, m in enumerate(ms):
                        for ch in range(2):
                            nc.tensor.matmul(
                                pts[(mi, ch)], w1[:, mi * 128:(mi + 1) * 128],
                                mhat[k][:, ch * 512:(ch + 1) * 512],
                                start=(k == 0), stop=(k == KT - 1))
                for mi, m in enumerate(ms):
                    for ch in range(2):
                        nc.scalar.activation(
                            h1[m][:, ch * 512:(ch + 1) * 512], pts[(mi, ch)],
                            mybir.ActivationFunctionType.Gelu,
                            bias=b1[:, m:m + 1], scale=1.0)

        # ---- phase 7: fc2 + residual + out ----
        with ExitStack() as ph:
            f2_ps = ph.enter_context(tc.tile_pool(name="f2_ps", bufs=4, space="PSUM"))
            w2_pool = ph.enter_context(tc.tile_pool(name="w2", bufs=3))
            o_pool = ph.enter_context(tc.tile_pool(name="outT", bufs=2))
            for ms in ([0, 1], [2, 3], [4, 5]):
                pts = {}
                for mi, m in enumerate(ms):
                    for ch in range(2):
                        pts[(mi, ch)] = f2_ps.tile([128, 512], F32, tag="f2ps", name=f"f2ps_{m}_{ch}")
                for k in range(24):
                    w2 = w2_pool.tile([128, 256], BF16, tag="w2")
                    nc.sync.dma_start(
                        out=w2, in_=d["w2"][k * 128:(k + 1) * 128,
                                            ms[0] * 128:(ms[-1] + 1) * 128])
                    for mi, m in enumerate(ms):
                        for ch in range(2):
                            nc.tensor.matmul(
                                pts[(mi, ch)], w2[:, mi * 128:(mi + 1) * 128],
                                h1[k][:, ch * 512:(ch + 1) * 512],
                                start=(k == 0), stop=(k == 23))
                for mi, m in enumerate(ms):
                    ot = o_pool.tile([128, NTOK], F32, tag="ot")
                    for ch in range(2):
                        nc.vector.scalar_tensor_tensor(
                            out=ot[:, ch * 512:(ch + 1) * 512],
                            in0=pts[(mi, ch)], scalar=b2[:, m:m + 1],
                            in1=x2[m][:, ch * 512:(ch + 1) * 512],
                            op0=mybir.AluOpType.add, op1=mybir.AluOpType.add)
                    nc.sync.dma_start(out=d["outT"][m * 128:(m + 1) * 128, :], in_=ot)


def _build(loop_n=None):
    nc = bacc.Bacc("TRN2", target_bir_lowering=False, debug=False, num_devices=8)
    dd = {}

    def din(name, shape):
        dd[name] = nc.dram_tensor(name, list(shape), F32, kind="ExternalInput").ap()

    din("xT", (DIM, NTOK))
    
    din("baug", (128, 18))
    
    
    
    din("bp", (128, 6))
    
    din("b1", (128, 24))
    
    din("b2", (128, 6))
    
    dd["waug"] = nc.dram_tensor("waug", [DIM, 2304], BF16, kind="ExternalInput").ap()
    dd["wv"] = nc.dram_tensor("wv", [DIM, DIM], BF16, kind="ExternalInput").ap()
    dd["wp"] = nc.dram_tensor("wp", [DIM, DIM], BF16, kind="ExternalInput").ap()
    dd["w1"] = nc.dram_tensor("w1", [DIM, 3072], BF16, kind="ExternalInput").ap()
    dd["bv"] = nc.dram_tensor("bv", [1, DIM], BF16, kind="ExternalInput").ap()
    dd["w2"] = nc.dram_tensor("w2", [3072, DIM], BF16, kind="ExternalInput").ap()
    dd["e2"] = nc.dram_tensor("e2", [64, NTOK], BF16, kind="ExternalInput").ap()
    dd["sel"] = nc.dram_tensor("sel", [128, 8 * 128], BF16, kind="ExternalInput").ap()
    dd["outT"] = nc.dram_tensor("outT", [DIM, NTOK], F32, kind="ExternalOutput").ap()

    with tile.TileContext(nc) as tc:
        if loop_n is None:
            _emit(nc, tc, dd)
        else:
            with tc.For_i(0, loop_n, 1):
                _emit(nc, tc, dd)
    nc.compile()
    return nc


# ---------------------------------------------------------------------------
# host side
# ---------------------------------------------------------------------------

def _col_tiles(b):
    """(n*128,) bias -> (128, n) column-tile layout."""
    n = b.shape[0] // 128
    return np.ascontiguousarray(b.reshape(n, 128).T)


def prep_weights(inputs):
    g = {k: np.asarray(v, np.float32) for k, v in inputs.items()}
    qkv_w, qkv_b = g["qkv_w"], g["qkv_b"]
    ln1_w, ln1_b = g["ln1_w"], g["ln1_b"]
    Wf = qkv_w * ln1_w[None, :]
    bf = qkv_b + qkv_w @ ln1_b
    Wq, bq = Wf[0:768], bf[0:768]
    Wk, bk = Wf[768:1536], bf[768:1536]
    Wv, bv = Wf[1536:2304], bf[1536:2304]
    rel = (g["rel_pos_d"], g["rel_pos_h"], g["rel_pos_w"])
    W_aug = np.zeros((2304, 768), np.float32)
    b_aug = np.zeros((2304,), np.float32)
    W_aug[0:768] = Wq * SCALE
    b_aug[0:768] = bq * SCALE
    W_aug[768:1536] = Wk
    b_aug[768:1536] = bk
    for h in range(NH):
        Wq_h, bq_h = Wq[h * 64:(h + 1) * 64], bq[h * 64:(h + 1) * 64]
        for ti in range(3):
            T = rel[ti][::-1]
            rows = 1536 + h * 64 + ti * 15
            W_aug[rows:rows + 15] = T @ Wq_h
            b_aug[rows:rows + 15] = T @ bq_h
    m = np.arange(512)
    # khat rows: 0-7 e_h, 8-15 e_d, 32-39 e_w, 64-127 k, zeros elsewhere
    E = np.zeros((64, 512), np.float32)
    E[(m // 8) % 8, m] = 1.0
    E[8 + m // 64, m] = 1.0
    E[32 + m % 8, m] = 1.0
    # one-hot row-selector matrices: out row (dp*32+j) of variant
    # v = hh*6 + ti*2 + g selects psb row hh*64 + ti*15 + 7 - (g*4+dp) + j
    sel = np.zeros((128, 8 * 128), np.float32)
    for hh in range(2):
        for ti in (1, 2):
            for gg in range(2):
                v = hh * 4 + (ti - 1) * 2 + gg
                for dp in range(4):
                    for j in range(8):
                        r = hh * 64 + ti * 15 + 7 - (gg * 4 + dp) + j
                        sel[r, v * 128 + dp * 32 + j] = 1.0
    return {
        "sel": np.ascontiguousarray(sel).astype(ml_dtypes.bfloat16),
        "waug": np.ascontiguousarray(W_aug.T).astype(ml_dtypes.bfloat16),
        "baug": _col_tiles(b_aug),
        "wv": np.ascontiguousarray(Wv.T).astype(ml_dtypes.bfloat16),
        "bv": np.ascontiguousarray(bv[None, :]).astype(ml_dtypes.bfloat16),
        "wp": np.ascontiguousarray(g["proj_w"].T).astype(ml_dtypes.bfloat16),
        "bp": _col_tiles(g["proj_b"]),
        "w1": np.ascontiguousarray((g["fc1_w"] * g["ln2_w"][None, :]).T).astype(ml_dtypes.bfloat16),
        "b1": _col_tiles(g["fc1_b"] + g["fc1_w"] @ g["ln2_b"]),
        "w2": np.ascontiguousarray(g["fc2_w"].T).astype(ml_dtypes.bfloat16),
        "b2": _col_tiles(g["fc2_b"]),
        "e2": np.ascontiguousarray(np.concatenate([E, E], axis=1)).astype(ml_dtypes.bfloat16),
    }


def shard_x(x):
    """x (B,D,H,W,C) -> list of 8 per-core (768, 1024) C-layout arrays."""
    B, D, H, W, C = x.shape
    win = x.reshape(B, D // WS, WS, H // WS, WS, W // WS, WS, C)
    win = win.transpose(0, 1, 3, 5, 2, 4, 6, 7).reshape(-1, WS ** 3, C)
    return [np.ascontiguousarray(win[2 * c:2 * c + 2].reshape(NTOK, C).T)
            for c in range(8)]


def unshard_out(outs, shape):
    B, D, H, W, C = shape
    full = np.concatenate([o.T for o in outs], axis=0).reshape(16, WS ** 3, C)
    x = full.reshape(B, D // WS, H // WS, W // WS, WS, WS, WS, C)
    x = x.transpose(0, 1, 4, 2, 5, 3, 6, 7).reshape(B, D, H, W, C)
    return np.ascontiguousarray(x)


_STATE = {}
PHASE_LIMIT = None  # "attn": stop after attention (diagnostics)


def _make_runner(nc):
    """Wrap a compiled Bass program in a persistent jitted SPMD callable."""
    import jax
    from jax.sharding import Mesh, PartitionSpec
    from jax.experimental.shard_map import shard_map
    from concourse import bass2jax

    bass2jax.install_neuronx_cc_hook()

    n_cores = 8
    partition_name = nc.partition_id_tensor.name if nc.partition_id_tensor else None
    in_names, out_names, out_avals, zero_outs = [], [], [], []
    for alloc in nc.m.functions[0].allocations:
        if not isinstance(alloc, mybir.MemoryLocationSet):
            continue
        name = alloc.memorylocations[0].name
        if alloc.kind == "ExternalInput":
            if name != partition_name:
                in_names.append(name)
        elif alloc.kind == "ExternalOutput":
            out_names.append(name)
            shape = tuple(alloc.tensor_shape)
            dtype = mybir.dt.np(alloc.dtype)
            out_avals.append(jax.core.ShapedArray(shape, dtype))
            zero_outs.append(np.zeros(shape, dtype))
    n_params = len(in_names)
    all_in_names = in_names + out_names
    if partition_name is not None:
        all_in_names = all_in_names + [partition_name]

    def _body(*args):
        operands = list(args)
        if partition_name is not None:
            operands.append(bass2jax.partition_id_tensor())
        outs = bass2jax._bass_exec_p.bind(
            *operands,
            out_avals=tuple(out_avals),
            in_names=tuple(all_in_names),
            out_names=tuple(out_names),
            lowering_input_output_aliases=(),
            sim_require_finite=True,
            sim_require_nnan=True,
            nc=nc,
        )
        return tuple(outs)

    devices = jax.devices()[:n_cores]
    mesh = Mesh(np.asarray(devices), ("core",))
    donate = tuple(range(n_params, n_params + len(out_names)))
    sharded = jax.jit(
        shard_map(_body, mesh=mesh,
                  in_specs=(PartitionSpec("core"),) * (n_params + len(out_names)),
                  out_specs=(PartitionSpec("core"),) * len(out_names)),
        donate_argnums=donate, keep_unused=True)

    def run(in_maps):
        per_core = [[np.asarray(m[nm]) for nm in in_names] for m in in_maps]
        concat_in = [np.concatenate([per_core[c][i] for c in range(n_cores)], axis=0)
                     for i in range(n_params)]
        concat_zero = [np.zeros((n_cores * z.shape[0], *z.shape[1:]), z.dtype)
                       for z in zero_outs]
        out_arrs = sharded(*concat_in, *concat_zero)
        return [
            {nm: np.asarray(out_arrs[i]).reshape(n_cores, *out_avals[i].shape)[c]
             for i, nm in enumerate(out_names)}
            for c in range(n_cores)
        ]

    return run, dict(sharded=sharded, body=_body, in_names=in_names,
                     out_names=out_names, out_avals=out_avals,
                     zero_outs=zero_outs, mesh=mesh, n_params=n_params)


def _get_runner():
    if "run" not in _STATE:
        run, internals = _make_runner(_build())
        _STATE["run"] = run
        _STATE["internals"] = internals
    return _STATE["run"]


def kernel(**inputs):
    x = np.asarray(inputs["x"], np.float32)
    w = prep_weights(inputs)
    shards = shard_x(x)
    in_maps = [dict(w, xT=shards[c]) for c in range(8)]
    run = _get_runner()
    results = run(in_maps)
    outs = [results[c]["outT"] for c in range(8)]
    return unshard_out(outs, x.shape)
